# revision 19
# baseline (speedup 1.0000x reference)
"""Trainium2 Bass kernel for 3-layer GRU (B=64,S=512,IN=64,H=512) + FC head.

Strategy: data-parallel over batch across 8 NeuronCores (8 samples/core).
Per core, layers run sequentially; per layer:
  phase A: gx = x_or_h @ W_ih.T + biases, all timesteps batched (PE, bf16)
  phase B: 512 sequential GRU steps. Per step the r/z PSUM tiles are
           pre-seeded with gx_rz (identity matmul) and the n tile with
           b_hh_n (rank-1 matmul) — both independent of h, so the PE does
           them inside the gate-phase gap of the previous step — then
           accumulates W_hh.T gh (bf16) in r,z,n order into three separate
           PSUM tiles so each sigmoid starts as soon as its slice is done.
           Update uses h' = (1-z)*n + z*h with 1-z computed on DVE.
           h state is bf16; h.T for the next step's stationary operand
           comes from 4 PE transposes.
Final FC uses the last step's h.T tiles directly.
"""

import sys

for p in ("/opt/trn_rl_repo",):
    if p not in sys.path:
        sys.path.insert(0, p)

import numpy as np
import ml_dtypes

import concourse.bass as bass
import concourse.tile as tile
from concourse import mybir
from concourse.bass_utils import run_bass_kernel_spmd

BF16 = ml_dtypes.bfloat16

B, S, IN, H, L, T_OUT = 64, 512, 64, 512, 3, 24
G = 3 * H          # 1536
NC = 8             # cores
BL = B // NC       # 8 samples per core
TOK = BL * S       # 4096 tokens per core
KC = H // 128      # 4 K-chunks

F32 = mybir.dt.float32
B16 = mybir.dt.bfloat16


def _split_sync_waits(nc, max_waits=1):
    """The nix walrus in this container rejects instructions carrying more
    than a couple of sync waits; split overflow waits onto preceding NOPs."""
    import bass_rust

    ctr = [0]
    for f in nc.m.functions:
        for blk in f.blocks:
            insts = blk.instructions
            i = 0
            while i < len(insts):
                inst = insts[i]
                si = inst.sync_info
                waits = list(si.on_wait) if (si and si.on_wait) else []
                if len(waits) > max_waits:
                    extra, keep = waits[:-max_waits], waits[-max_waits:]
                    nops = []
                    while extra:
                        chunk, extra = extra[:max_waits], extra[max_waits:]
                        ctr[0] += 1
                        nop = bass_rust.InstNoOp(
                            name=f"I-waitsplit-{ctr[0]}", ins=[], outs=[]
                        )
                        nop.engine = inst.engine
                        nop.sync_info = bass_rust.SyncInfo(
                            on_wait=chunk, on_update=[]
                        )
                        nops.append(nop)
                    inst.sync_info = bass_rust.SyncInfo(
                        on_wait=keep,
                        on_update=list(si.on_update) if si.on_update else [],
                    )
                    for j, nop in enumerate(nops):
                        insts.insert(i + j, nop)
                    i += len(nops)
                i += 1


def build_bass(s_steps=S, dbg=False):
    nc = bass.Bass(
        trn_type="TRN2", target_bir_lowering=False, debug=False, num_devices=NC
    )

    # ---- dram I/O ----
    d_xT = nc.dram_tensor("xT", [IN, BL * s_steps], B16, kind="ExternalInput")
    d_whhT = [
        nc.dram_tensor(f"whhT{l}", [H, G], B16, kind="ExternalInput")
        for l in range(L)
    ]
    d_wihT = [nc.dram_tensor("wihT0", [IN, G], B16, kind="ExternalInput")] + [
        nc.dram_tensor(f"wihT{l}", [H, G], B16, kind="ExternalInput")
        for l in (1, 2)
    ]
    d_gxbias = [
        nc.dram_tensor(f"gxbias{l}", [128, G], B16, kind="ExternalInput")
        for l in range(L)
    ]
    d_bhhn = [
        nc.dram_tensor(f"bhhn{l}", [1, H], B16, kind="ExternalInput")
        for l in range(L)
    ]
    d_ones = nc.dram_tensor("ones18", [1, BL], B16, kind="ExternalInput")
    d_id8 = nc.dram_tensor("ident8", [BL, BL], B16, kind="ExternalInput")
    d_fcw = nc.dram_tensor("fcwT", [H, T_OUT], B16, kind="ExternalInput")
    d_fcb = nc.dram_tensor("fcb", [1, T_OUT], B16, kind="ExternalInput")
    d_out = nc.dram_tensor("out", [BL, T_OUT], F32, kind="ExternalOutput")

    ntok = BL * s_steps
    nblk = ntok // 128  # 128-token blocks in gx phase

    with tile.TileContext(nc) as tc:
        with (
            tc.tile_pool(name="const", bufs=1) as cpool,
            tc.tile_pool(name="wl", bufs=1) as wlpool,
            tc.tile_pool(name="state", bufs=1) as spool,
            tc.tile_pool(name="gxin", bufs=4) as gxin_pool,
            tc.tile_pool(name="gxout", bufs=2) as gxout_pool,
            tc.tile_pool(name="gates", bufs=2) as gpool,
            tc.tile_pool(name="mm", bufs=2, space="PSUM") as mmpool,
            tc.tile_pool(name="tp", bufs=1, space="PSUM") as tppool,
            tc.tile_pool(name="dm", bufs=1, space="PSUM") as dmpool,
            tc.tile_pool(name="dram", bufs=1, space="DRAM") as dpool,
        ):
            # ---- constants resident all run ----
            xT = cpool.tile([IN, ntok], B16, tag="xT")
            nc.sync.dma_start(xT[:], d_xT.ap())
            ones18 = cpool.tile([1, BL], B16, tag="ones")
            nc.sync.dma_start(ones18[:], d_ones.ap())
            id8 = cpool.tile([BL, BL], B16, tag="id8")
            nc.sync.dma_start(id8[:], d_id8.ap())
            fcw = cpool.tile([128, KC, T_OUT], B16, tag="fcw")
            nc.sync.dma_start(
                fcw[:],
                d_fcw.ap().rearrange("(k p) t -> p k t", p=128),
            )
            fcb = cpool.tile([1, T_OUT], B16, tag="fcb")
            nc.sync.dma_start(fcb[:], d_fcb.ap())

            # history of h^{l}.T for next layer's gx ([128, k, t, b])
            hist = cpool.tile([128, KC, s_steps, BL], B16, tag="hist")

            # persistent state
            curhT = spool.tile([128, KC * BL], B16, tag="curhT")
            h_st = spool.tile([BL, H], B16, tag="h")

            # per-layer weights (reloaded per layer)
            whh = wlpool.tile([128, KC, G], B16, tag="whh")
            wih = wlpool.tile([128, KC, G], B16, tag="wih")
            gxbias = wlpool.tile([128, G], B16, tag="gxbias")
            bhhn = wlpool.tile([1, H], B16, tag="bhhn")

            gx_dram = dpool.tile([ntok, G], B16, tag="gx")
            junk_dram = dpool.tile([BL, 512], B16, tag="junk")

            # HAM filler: the PE idles ~2-3us per step while the gate chain
            # runs, which re-throttles the clock to K=4/8 for the following
            # matmul burst (one oscillation per step, ~2x on every matmul).
            # Dummy accumulating matmuls during the gap keep the activity
            # monitor fed so the real matmuls run at 2.4 GHz.
            dummy_ps = dmpool.tile([128, 512], F32, tag="dm")
            dummy_started = [False]

            def pe_filler(n_mm):
                # full-width (M=128, N=512) matmuls on resident tiles: max
                # array activity per instruction, no data dependencies.
                for i in range(n_mm):
                    nc.tensor.matmul(
                        dummy_ps[:],
                        whh[:, 0, 0:128],
                        gxbias[:, 512:1024],
                        start=(not dummy_started[0]),
                        stop=False,
                        skip_group_check=True,
                    )
                    dummy_started[0] = True

            for layer in range(L):
                # ---------- load layer weights ----------
                nc.sync.dma_start(
                    whh[:],
                    d_whhT[layer].ap().rearrange("(k p) g -> p k g", p=128),
                )
                if layer == 0:
                    nc.sync.dma_start(wih[0:IN, 0, :], d_wihT[0].ap())
                else:
                    nc.sync.dma_start(
                        wih[:],
                        d_wihT[layer].ap().rearrange("(k p) g -> p k g", p=128),
                    )
                nc.sync.dma_start(gxbias[:], d_gxbias[layer].ap())
                nc.sync.dma_start(bhhn[:], d_bhhn[layer].ap())

                # ---------- phase A: gx for all tokens ----------
                kc_in = 1 if layer == 0 else KC
                for blk in range(nblk):
                    gxo = gxout_pool.tile([128, G], B16, tag="gxo")
                    for s3 in range(3):
                        ps = mmpool.tile([128, 512], F32, tag=f"m{s3}")
                        for k in range(kc_in):
                            if layer == 0:
                                lhsT = xT[:, blk * 128 : (blk + 1) * 128]
                                rhs = wih[0:IN, 0, s3 * 512 : (s3 + 1) * 512]
                            else:
                                lhsT = hist[:, k, blk * 16 : (blk + 1) * 16, :]
                                rhs = wih[:, k, s3 * 512 : (s3 + 1) * 512]
                            nc.tensor.matmul(
                                ps[:],
                                lhsT,
                                rhs,
                                start=(k == 0),
                                stop=(k == kc_in - 1),
                            )
                        nc.vector.tensor_add(
                            gxo[:, s3 * 512 : (s3 + 1) * 512],
                            ps[:],
                            gxbias[:, s3 * 512 : (s3 + 1) * 512],
                        )
                    nc.sync.dma_start(
                        gx_dram[blk * 128 : (blk + 1) * 128, :], gxo[:]
                    )

                # ---------- phase B: recurrence ----------
                nc.scalar.memzero(h_st[:])

                def hT_flush(t):
                    # transpose h_{t} (bf16) -> psum, record as stationary
                    # operand for step t+1 and into the gx history.
                    pst = tppool.tile([128, KC * BL], B16, tag="tp")
                    for k in range(KC):
                        nc.tensor.transpose(
                            pst[:, k * BL : (k + 1) * BL],
                            h_st[:, k * 128 : (k + 1) * 128],
                            id8[:],
                        )
                    nc.scalar.copy(curhT[:], pst[:])
                    if layer < L - 1 and t >= 0:
                        nc.vector.tensor_copy(
                            hist[:, :, t, :],
                            pst[:].rearrange("p (k b) -> p k b", k=KC),
                        )

                hT_flush(-1)  # curhT <- 0

                for t in range(s_steps):
                    gxt = gxin_pool.tile([BL, G], B16, tag="gxt")
                    nc.sync.dma_start(
                        gxt[:], gx_dram[t * BL : (t + 1) * BL, :]
                    )
                    ps_r = mmpool.tile([BL, 512], F32, tag="m0")
                    ps_z = mmpool.tile([BL, 512], F32, tag="m1")
                    ps_n = mmpool.tile([BL, 512], F32, tag="m2")
                    # seed psum: r,z tiles get gx (identity matmul), n tile
                    # gets b_hh_n (rank-1).  All independent of h_{t-1}, so
                    # these run inside the previous step's gate-phase gap.
                    nc.tensor.matmul(
                        ps_r[:], id8[:], gxt[:, 0:512],
                        start=True, stop=False, skip_group_check=True,
                    )
                    nc.tensor.matmul(
                        ps_z[:], id8[:], gxt[:, 512:1024],
                        start=True, stop=False, skip_group_check=True,
                    )
                    nc.tensor.matmul(
                        ps_n[:], ones18[:], bhhn[:],
                        start=True, stop=False, skip_group_check=True,
                    )
                    # transpose of h_{t-1} emitted here: in PE program order
                    # the seeds above precede it, filling the PE gap while
                    # step t-1's gates finish.
                    pe_filler(10)
                    if t > 0:
                        hT_flush(t - 1)
                    # gh accumulate, r -> z -> n so sigmoid_r starts early
                    for s3, ps in ((0, ps_r), (1, ps_z), (2, ps_n)):
                        for k in range(KC):
                            nc.tensor.matmul(
                                ps[:],
                                curhT[:, k * BL : (k + 1) * BL],
                                whh[:, k, s3 * 512 : (s3 + 1) * 512],
                                start=False,
                                stop=(k == KC - 1),
                                skip_group_check=True,
                            )
                    # gates
                    rz = gpool.tile([BL, 2 * H], B16, tag="rz")
                    nc.scalar.activation(
                        rz[:, 0:H], ps_r[:],
                        mybir.ActivationFunctionType.Sigmoid,
                    )
                    nc.scalar.activation(
                        rz[:, H : 2 * H], ps_z[:],
                        mybir.ActivationFunctionType.Sigmoid,
                    )
                    t1 = gpool.tile([BL, H], B16, tag="t1")
                    nc.vector.tensor_mul(t1[:], rz[:, 0:H], ps_n[:])
                    t2 = gpool.tile([BL, H], B16, tag="t2")
                    nc.vector.tensor_add(t2[:], t1[:], gxt[:, 2 * 512 : 3 * 512])
                    nt = gpool.tile([BL, H], B16, tag="nt")
                    nc.scalar.activation(
                        nt[:], t2[:], mybir.ActivationFunctionType.Tanh
                    )
                    # h' = n + z*(h - n)
                    dd = gpool.tile([BL, H], B16, tag="dd")
                    nc.vector.tensor_sub(dd[:], h_st[:], nt[:])
                    t3 = gpool.tile([BL, H], B16, tag="t3")
                    nc.vector.tensor_mul(t3[:], rz[:, H : 2 * H], dd[:])
                    nc.vector.tensor_add(h_st[:], t3[:], nt[:])

                # final h.T (records hist[s-1] and leaves curhT = h_final)
                hT_flush(s_steps - 1)

            # anti-DCE reader for the HAM-filler psum: one copy + DMA out
            nc.tensor.matmul(
                dummy_ps[:], whh[:, 0, 0:128], gxbias[:, 0:512],
                start=False, stop=True, skip_group_check=True,
            )
            junk_sb = gpool.tile([BL, 512], B16, tag="junk")
            nc.scalar.copy(junk_sb[:], dummy_ps[0:BL, :])
            nc.sync.dma_start(junk_dram[:], junk_sb[:])

            # ---------- FC head ----------
            psfull = mmpool.tile([BL, 512], F32, tag="m0")
            psf = psfull[:, 0:T_OUT]
            nc.tensor.matmul(
                psf,
                ones18[:],
                fcb[:],
                start=True,
                stop=False,
                skip_group_check=True,
            )
            for k in range(KC):
                nc.tensor.matmul(
                    psf,
                    curhT[:, k * BL : (k + 1) * BL],
                    fcw[:, k, :],
                    start=False,
                    stop=(k == KC - 1),
                    skip_group_check=True,
                )
            out_sb = gpool.tile([BL, T_OUT], F32, tag="osb")
            nc.scalar.copy(out_sb[:], psf)
            nc.sync.dma_start(d_out.ap(), out_sb[:])

    _split_sync_waits(nc)
    return nc


_CACHE = {}


def _get_bass(s_steps):
    if s_steps not in _CACHE:
        _CACHE[s_steps] = build_bass(s_steps)
    return _CACHE[s_steps]


def make_in_maps(inputs, s_steps=S):
    x = np.asarray(inputs["x"], np.float32)
    common = {}
    for l in range(L):
        whh = np.asarray(inputs[f"w_hh_l{l}"], np.float32)  # [G, H]
        common[f"whhT{l}"] = np.ascontiguousarray(whh.T).astype(BF16)
        wih = np.asarray(inputs[f"w_ih_l{l}"], np.float32)  # [G, in]
        common[f"wihT{l}" if l else "wihT0"] = np.ascontiguousarray(
            wih.T
        ).astype(BF16)
        b_ih = np.asarray(inputs[f"b_ih_l{l}"], np.float32)
        b_hh = np.asarray(inputs[f"b_hh_l{l}"], np.float32)
        gb = b_ih.copy()
        gb[: 2 * H] += b_hh[: 2 * H]
        common[f"gxbias{l}"] = np.broadcast_to(
            gb.astype(BF16), (128, G)
        ).copy()
        common[f"bhhn{l}"] = b_hh[2 * H :].reshape(1, H).astype(BF16)
    common["ones18"] = np.ones((1, BL), BF16)
    common["ident8"] = np.eye(BL, dtype=BF16)
    common["fcwT"] = np.ascontiguousarray(
        np.asarray(inputs["fc_w"], np.float32).T
    ).astype(BF16)  # [H, T_OUT]
    common["fcb"] = np.asarray(inputs["fc_b"], np.float32).reshape(
        1, T_OUT
    ).astype(BF16)

    in_maps = []
    for c in range(NC):
        xs = x[c * BL : (c + 1) * BL, :s_steps, :]  # [BL, s, IN]
        xT = np.ascontiguousarray(xs.transpose(2, 1, 0)).reshape(
            IN, s_steps * BL
        )  # col = t*BL+b
        m = dict(common)
        m["xT"] = xT.astype(BF16)
        in_maps.append(m)
    return in_maps


def assemble_output(res) -> np.ndarray:
    out = np.concatenate(
        [res.results[c]["out"] for c in range(NC)], axis=0
    )
    return out.astype(np.float32)


def kernel(**inputs) -> np.ndarray:
    nc = _get_bass(S)
    in_maps = make_in_maps(inputs, S)
    res = run_bass_kernel_spmd(nc, in_maps, core_ids=list(range(NC)))
    return assemble_output(res)


# revision 20
# speedup vs baseline: 1.0455x; 1.0455x over previous
"""Trainium2 Bass kernel for 3-layer GRU (B=64,S=512,IN=64,H=512) + FC head.

Strategy: data-parallel over batch across 8 NeuronCores (8 samples/core).
Per core, layers run sequentially; per layer:
  phase A: gx = x_or_h @ W_ih.T + biases, all timesteps batched (PE, bf16)
  phase B: 512 sequential GRU steps. Per step the r/z PSUM tiles are
           pre-seeded with gx_rz (identity matmul) and the n tile with
           b_hh_n (rank-1 matmul) — both independent of h, so the PE does
           them inside the gate-phase gap of the previous step — then
           accumulates W_hh.T gh (bf16) in r,z,n order into three separate
           PSUM tiles so each sigmoid starts as soon as its slice is done.
           Update uses h' = (1-z)*n + z*h with 1-z computed on DVE.
           h state is bf16; h.T for the next step's stationary operand
           comes from 4 PE transposes.
Final FC uses the last step's h.T tiles directly.
"""

import sys

for p in ("/opt/trn_rl_repo",):
    if p not in sys.path:
        sys.path.insert(0, p)

import numpy as np
import ml_dtypes

import concourse.bass as bass
import concourse.tile as tile
from concourse import mybir
from concourse.bass_utils import run_bass_kernel_spmd

BF16 = ml_dtypes.bfloat16

B, S, IN, H, L, T_OUT = 64, 512, 64, 512, 3, 24
G = 3 * H          # 1536
NC = 8             # cores
BL = B // NC       # 8 samples per core
TOK = BL * S       # 4096 tokens per core
KC = H // 128      # 4 K-chunks

F32 = mybir.dt.float32
B16 = mybir.dt.bfloat16


def _split_sync_waits(nc, max_waits=1):
    """The nix walrus in this container rejects instructions carrying more
    than a couple of sync waits; split overflow waits onto preceding NOPs."""
    import bass_rust

    ctr = [0]
    for f in nc.m.functions:
        for blk in f.blocks:
            insts = blk.instructions
            i = 0
            while i < len(insts):
                inst = insts[i]
                si = inst.sync_info
                waits = list(si.on_wait) if (si and si.on_wait) else []
                if len(waits) > max_waits:
                    extra, keep = waits[:-max_waits], waits[-max_waits:]
                    nops = []
                    while extra:
                        chunk, extra = extra[:max_waits], extra[max_waits:]
                        ctr[0] += 1
                        nop = bass_rust.InstNoOp(
                            name=f"I-waitsplit-{ctr[0]}", ins=[], outs=[]
                        )
                        nop.engine = inst.engine
                        nop.sync_info = bass_rust.SyncInfo(
                            on_wait=chunk, on_update=[]
                        )
                        nops.append(nop)
                    inst.sync_info = bass_rust.SyncInfo(
                        on_wait=keep,
                        on_update=list(si.on_update) if si.on_update else [],
                    )
                    for j, nop in enumerate(nops):
                        insts.insert(i + j, nop)
                    i += len(nops)
                i += 1


def build_bass(s_steps=S, dbg=False):
    nc = bass.Bass(
        trn_type="TRN2", target_bir_lowering=False, debug=False, num_devices=NC
    )

    # ---- dram I/O ----
    d_xT = nc.dram_tensor("xT", [IN, BL * s_steps], B16, kind="ExternalInput")
    d_whhT = [
        nc.dram_tensor(f"whhT{l}", [H, G], B16, kind="ExternalInput")
        for l in range(L)
    ]
    d_wihT = [nc.dram_tensor("wihT0", [IN, G], B16, kind="ExternalInput")] + [
        nc.dram_tensor(f"wihT{l}", [H, G], B16, kind="ExternalInput")
        for l in (1, 2)
    ]
    d_gxbias = [
        nc.dram_tensor(f"gxbias{l}", [128, G], B16, kind="ExternalInput")
        for l in range(L)
    ]
    d_bhhn = [
        nc.dram_tensor(f"bhhn{l}", [1, H], B16, kind="ExternalInput")
        for l in range(L)
    ]
    d_ones = nc.dram_tensor("ones18", [1, BL], B16, kind="ExternalInput")
    d_id8 = nc.dram_tensor("ident8", [BL, BL], B16, kind="ExternalInput")
    d_fcw = nc.dram_tensor("fcwT", [H, T_OUT], B16, kind="ExternalInput")
    d_fcb = nc.dram_tensor("fcb", [1, T_OUT], B16, kind="ExternalInput")
    d_out = nc.dram_tensor("out", [BL, T_OUT], F32, kind="ExternalOutput")

    ntok = BL * s_steps
    nblk = ntok // 128  # 128-token blocks in gx phase

    with tile.TileContext(nc) as tc:
        with (
            tc.tile_pool(name="const", bufs=1) as cpool,
            tc.tile_pool(name="wl", bufs=1) as wlpool,
            tc.tile_pool(name="state", bufs=1) as spool,
            tc.tile_pool(name="gxin", bufs=4) as gxin_pool,
            tc.tile_pool(name="gxout", bufs=2) as gxout_pool,
            tc.tile_pool(name="gates", bufs=2) as gpool,
            tc.tile_pool(name="mm", bufs=2, space="PSUM") as mmpool,
            tc.tile_pool(name="tp", bufs=1, space="PSUM") as tppool,
            tc.tile_pool(name="dm", bufs=1, space="PSUM") as dmpool,
            tc.tile_pool(name="dram", bufs=1, space="DRAM") as dpool,
        ):
            # ---- constants resident all run ----
            xT = cpool.tile([IN, ntok], B16, tag="xT")
            nc.sync.dma_start(xT[:], d_xT.ap())
            ones18 = cpool.tile([1, BL], B16, tag="ones")
            nc.sync.dma_start(ones18[:], d_ones.ap())
            id8 = cpool.tile([BL, BL], B16, tag="id8")
            nc.sync.dma_start(id8[:], d_id8.ap())
            fcw = cpool.tile([128, KC, T_OUT], B16, tag="fcw")
            nc.sync.dma_start(
                fcw[:],
                d_fcw.ap().rearrange("(k p) t -> p k t", p=128),
            )
            fcb = cpool.tile([1, T_OUT], B16, tag="fcb")
            nc.sync.dma_start(fcb[:], d_fcb.ap())

            # history of h^{l}.T for next layer's gx ([128, k, t, b])
            hist = cpool.tile([128, KC, s_steps, BL], B16, tag="hist")

            # persistent state
            curhT = spool.tile([128, KC * BL], B16, tag="curhT")
            h_st = spool.tile([BL, H], B16, tag="h")

            # per-layer weights (reloaded per layer)
            whh = wlpool.tile([128, KC, G], B16, tag="whh")
            wih = wlpool.tile([128, KC, G], B16, tag="wih")
            gxbias = wlpool.tile([128, G], B16, tag="gxbias")
            bhhn = wlpool.tile([1, H], B16, tag="bhhn")

            gx_dram = dpool.tile([ntok, G], B16, tag="gx")
            junk_dram = dpool.tile([BL, 512], B16, tag="junk")

            # HAM filler: the PE idles ~2-3us per step while the gate chain
            # runs, which re-throttles the clock to K=4/8 for the following
            # matmul burst (one oscillation per step, ~2x on every matmul).
            # Dummy accumulating matmuls during the gap keep the activity
            # monitor fed so the real matmuls run at 2.4 GHz.
            dummy_ps = dmpool.tile([128, 512], F32, tag="dm")
            dummy_started = [False]

            def pe_filler(n_mm):
                # full-width (M=128, N=512) matmuls on resident tiles: max
                # array activity per instruction, no data dependencies.
                for i in range(n_mm):
                    nc.tensor.matmul(
                        dummy_ps[:],
                        whh[:, 0, 0:128],
                        gxbias[:, 512:1024],
                        start=(not dummy_started[0]),
                        stop=False,
                        skip_group_check=True,
                    )
                    dummy_started[0] = True

            for layer in range(L):
                # ---------- load layer weights ----------
                nc.sync.dma_start(
                    whh[:],
                    d_whhT[layer].ap().rearrange("(k p) g -> p k g", p=128),
                )
                if layer == 0:
                    nc.sync.dma_start(wih[0:IN, 0, :], d_wihT[0].ap())
                else:
                    nc.sync.dma_start(
                        wih[:],
                        d_wihT[layer].ap().rearrange("(k p) g -> p k g", p=128),
                    )
                nc.sync.dma_start(gxbias[:], d_gxbias[layer].ap())
                nc.sync.dma_start(bhhn[:], d_bhhn[layer].ap())

                # ---------- phase A: gx for all tokens ----------
                kc_in = 1 if layer == 0 else KC
                for blk in range(nblk):
                    gxo = gxout_pool.tile([128, G], B16, tag="gxo")
                    for s3 in range(3):
                        ps = mmpool.tile([128, 512], F32, tag=f"m{s3}")
                        for k in range(kc_in):
                            if layer == 0:
                                lhsT = xT[:, blk * 128 : (blk + 1) * 128]
                                rhs = wih[0:IN, 0, s3 * 512 : (s3 + 1) * 512]
                            else:
                                lhsT = hist[:, k, blk * 16 : (blk + 1) * 16, :]
                                rhs = wih[:, k, s3 * 512 : (s3 + 1) * 512]
                            nc.tensor.matmul(
                                ps[:],
                                lhsT,
                                rhs,
                                start=(k == 0),
                                stop=(k == kc_in - 1),
                            )
                        nc.vector.tensor_add(
                            gxo[:, s3 * 512 : (s3 + 1) * 512],
                            ps[:],
                            gxbias[:, s3 * 512 : (s3 + 1) * 512],
                        )
                    nc.sync.dma_start(
                        gx_dram[blk * 128 : (blk + 1) * 128, :], gxo[:]
                    )

                # ---------- phase B: recurrence ----------
                nc.scalar.memzero(h_st[:])

                def hT_flush(t):
                    # transpose h_{t} (bf16) -> psum, record as stationary
                    # operand for step t+1 and into the gx history.
                    pst = tppool.tile([128, KC * BL], B16, tag="tp")
                    for k in range(KC):
                        nc.tensor.transpose(
                            pst[:, k * BL : (k + 1) * BL],
                            h_st[:, k * 128 : (k + 1) * 128],
                            id8[:],
                        )
                    nc.scalar.copy(curhT[:], pst[:])
                    if layer < L - 1 and t >= 0:
                        nc.vector.tensor_copy(
                            hist[:, :, t, :],
                            pst[:].rearrange("p (k b) -> p k b", k=KC),
                        )

                hT_flush(-1)  # curhT <- 0

                for t in range(s_steps):
                    gxt = gxin_pool.tile([BL, G], B16, tag="gxt")
                    nc.sync.dma_start(
                        gxt[:], gx_dram[t * BL : (t + 1) * BL, :]
                    )
                    ps_r = mmpool.tile([BL, 512], F32, tag="m0")
                    ps_z = mmpool.tile([BL, 512], F32, tag="m1")
                    ps_n = mmpool.tile([BL, 512], F32, tag="m2")
                    # seed psum: r,z tiles get gx (identity matmul), n tile
                    # gets b_hh_n (rank-1).  All independent of h_{t-1}, so
                    # these run inside the previous step's gate-phase gap.
                    nc.tensor.matmul(
                        ps_r[:], id8[:], gxt[:, 0:512],
                        start=True, stop=False, skip_group_check=True,
                    )
                    nc.tensor.matmul(
                        ps_z[:], id8[:], gxt[:, 512:1024],
                        start=True, stop=False, skip_group_check=True,
                    )
                    nc.tensor.matmul(
                        ps_n[:], ones18[:], bhhn[:],
                        start=True, stop=False, skip_group_check=True,
                    )
                    # transpose of h_{t-1} emitted here: in PE program order
                    # the seeds above precede it, filling the PE gap while
                    # step t-1's gates finish.
                    pe_filler(10)
                    if t > 0:
                        hT_flush(t - 1)
                    # gh accumulate, n -> r -> z: t1 needs ps_n and
                    # sigmoid_r, so those finish first; z overlaps t1/t2.
                    for s3, ps in ((2, ps_n), (0, ps_r), (1, ps_z)):
                        for k in range(KC):
                            nc.tensor.matmul(
                                ps[:],
                                curhT[:, k * BL : (k + 1) * BL],
                                whh[:, k, s3 * 512 : (s3 + 1) * 512],
                                start=False,
                                stop=(k == KC - 1),
                                skip_group_check=True,
                            )
                    # gates
                    rz = gpool.tile([BL, 2 * H], B16, tag="rz")
                    nc.scalar.activation(
                        rz[:, 0:H], ps_r[:],
                        mybir.ActivationFunctionType.Sigmoid,
                    )
                    nc.scalar.activation(
                        rz[:, H : 2 * H], ps_z[:],
                        mybir.ActivationFunctionType.Sigmoid,
                    )
                    t1 = gpool.tile([BL, H], B16, tag="t1")
                    nc.vector.tensor_mul(t1[:], rz[:, 0:H], ps_n[:])
                    t2 = gpool.tile([BL, H], B16, tag="t2")
                    nc.vector.tensor_add(t2[:], t1[:], gxt[:, 2 * 512 : 3 * 512])
                    bb = gpool.tile([BL, H], B16, tag="bb")
                    nc.vector.tensor_mul(bb[:], rz[:, H : 2 * H], h_st[:])
                    z2 = gpool.tile([BL, H], B16, tag="z2")
                    nc.scalar.activation(
                        z2[:], rz[:, H : 2 * H],
                        mybir.ActivationFunctionType.Copy,
                        bias=1.0, scale=-1.0,
                    )
                    nt = gpool.tile([BL, H], B16, tag="nt")
                    nc.scalar.activation(
                        nt[:], t2[:], mybir.ActivationFunctionType.Tanh
                    )
                    aa = gpool.tile([BL, H], B16, tag="aa")
                    nc.vector.tensor_mul(aa[:], z2[:], nt[:])
                    nc.vector.tensor_add(h_st[:], aa[:], bb[:])

                # final h.T (records hist[s-1] and leaves curhT = h_final)
                hT_flush(s_steps - 1)

            # anti-DCE reader for the HAM-filler psum: one copy + DMA out
            nc.tensor.matmul(
                dummy_ps[:], whh[:, 0, 0:128], gxbias[:, 0:512],
                start=False, stop=True, skip_group_check=True,
            )
            junk_sb = gpool.tile([BL, 512], B16, tag="junk")
            nc.scalar.copy(junk_sb[:], dummy_ps[0:BL, :])
            nc.sync.dma_start(junk_dram[:], junk_sb[:])

            # ---------- FC head ----------
            psfull = mmpool.tile([BL, 512], F32, tag="m0")
            psf = psfull[:, 0:T_OUT]
            nc.tensor.matmul(
                psf,
                ones18[:],
                fcb[:],
                start=True,
                stop=False,
                skip_group_check=True,
            )
            for k in range(KC):
                nc.tensor.matmul(
                    psf,
                    curhT[:, k * BL : (k + 1) * BL],
                    fcw[:, k, :],
                    start=False,
                    stop=(k == KC - 1),
                    skip_group_check=True,
                )
            out_sb = gpool.tile([BL, T_OUT], F32, tag="osb")
            nc.scalar.copy(out_sb[:], psf)
            nc.sync.dma_start(d_out.ap(), out_sb[:])

    _split_sync_waits(nc)
    return nc


_CACHE = {}


def _get_bass(s_steps):
    if s_steps not in _CACHE:
        _CACHE[s_steps] = build_bass(s_steps)
    return _CACHE[s_steps]


def make_in_maps(inputs, s_steps=S):
    x = np.asarray(inputs["x"], np.float32)
    common = {}
    for l in range(L):
        whh = np.asarray(inputs[f"w_hh_l{l}"], np.float32)  # [G, H]
        common[f"whhT{l}"] = np.ascontiguousarray(whh.T).astype(BF16)
        wih = np.asarray(inputs[f"w_ih_l{l}"], np.float32)  # [G, in]
        common[f"wihT{l}" if l else "wihT0"] = np.ascontiguousarray(
            wih.T
        ).astype(BF16)
        b_ih = np.asarray(inputs[f"b_ih_l{l}"], np.float32)
        b_hh = np.asarray(inputs[f"b_hh_l{l}"], np.float32)
        gb = b_ih.copy()
        gb[: 2 * H] += b_hh[: 2 * H]
        common[f"gxbias{l}"] = np.broadcast_to(
            gb.astype(BF16), (128, G)
        ).copy()
        common[f"bhhn{l}"] = b_hh[2 * H :].reshape(1, H).astype(BF16)
    common["ones18"] = np.ones((1, BL), BF16)
    common["ident8"] = np.eye(BL, dtype=BF16)
    common["fcwT"] = np.ascontiguousarray(
        np.asarray(inputs["fc_w"], np.float32).T
    ).astype(BF16)  # [H, T_OUT]
    common["fcb"] = np.asarray(inputs["fc_b"], np.float32).reshape(
        1, T_OUT
    ).astype(BF16)

    in_maps = []
    for c in range(NC):
        xs = x[c * BL : (c + 1) * BL, :s_steps, :]  # [BL, s, IN]
        xT = np.ascontiguousarray(xs.transpose(2, 1, 0)).reshape(
            IN, s_steps * BL
        )  # col = t*BL+b
        m = dict(common)
        m["xT"] = xT.astype(BF16)
        in_maps.append(m)
    return in_maps


def assemble_output(res) -> np.ndarray:
    out = np.concatenate(
        [res.results[c]["out"] for c in range(NC)], axis=0
    )
    return out.astype(np.float32)


def kernel(**inputs) -> np.ndarray:
    nc = _get_bass(S)
    in_maps = make_in_maps(inputs, S)
    res = run_bass_kernel_spmd(nc, in_maps, core_ids=list(range(NC)))
    return assemble_output(res)


# revision 21
# speedup vs baseline: 1.0796x; 1.0327x over previous
"""Trainium2 Bass kernel for 3-layer GRU (B=64,S=512,IN=64,H=512) + FC head.

Strategy: data-parallel over batch across 8 NeuronCores (8 samples/core).
Per core, layers run sequentially; per layer:
  phase A: gx = x_or_h @ W_ih.T + biases, all timesteps batched (PE, bf16)
  phase B: 512 sequential GRU steps. Per step the r/z PSUM tiles are
           pre-seeded with gx_rz (identity matmul) and the n tile with
           b_hh_n (rank-1 matmul) — both independent of h, so the PE does
           them inside the gate-phase gap of the previous step — then
           accumulates W_hh.T gh (bf16) in r,z,n order into three separate
           PSUM tiles so each sigmoid starts as soon as its slice is done.
           Update uses h' = (1-z)*n + z*h with 1-z computed on DVE.
           h state is bf16; h.T for the next step's stationary operand
           comes from 4 PE transposes.
Final FC uses the last step's h.T tiles directly.
"""

import sys

for p in ("/opt/trn_rl_repo",):
    if p not in sys.path:
        sys.path.insert(0, p)

import numpy as np
import ml_dtypes

import concourse.bass as bass
import concourse.tile as tile
from concourse import mybir
from concourse.bass_utils import run_bass_kernel_spmd

BF16 = ml_dtypes.bfloat16

B, S, IN, H, L, T_OUT = 64, 512, 64, 512, 3, 24
G = 3 * H          # 1536
NC = 8             # cores
BL = B // NC       # 8 samples per core
TOK = BL * S       # 4096 tokens per core
KC = H // 128      # 4 K-chunks

F32 = mybir.dt.float32
B16 = mybir.dt.bfloat16


def _split_sync_waits(nc, max_waits=1):
    """The nix walrus in this container rejects instructions carrying more
    than a couple of sync waits; split overflow waits onto preceding NOPs."""
    import bass_rust

    ctr = [0]
    for f in nc.m.functions:
        for blk in f.blocks:
            insts = blk.instructions
            i = 0
            while i < len(insts):
                inst = insts[i]
                si = inst.sync_info
                waits = list(si.on_wait) if (si and si.on_wait) else []
                if len(waits) > max_waits:
                    extra, keep = waits[:-max_waits], waits[-max_waits:]
                    nops = []
                    while extra:
                        chunk, extra = extra[:max_waits], extra[max_waits:]
                        ctr[0] += 1
                        nop = bass_rust.InstNoOp(
                            name=f"I-waitsplit-{ctr[0]}", ins=[], outs=[]
                        )
                        nop.engine = inst.engine
                        nop.sync_info = bass_rust.SyncInfo(
                            on_wait=chunk, on_update=[]
                        )
                        nops.append(nop)
                    inst.sync_info = bass_rust.SyncInfo(
                        on_wait=keep,
                        on_update=list(si.on_update) if si.on_update else [],
                    )
                    for j, nop in enumerate(nops):
                        insts.insert(i + j, nop)
                    i += len(nops)
                i += 1


def build_bass(s_steps=S, dbg=False):
    nc = bass.Bass(
        trn_type="TRN2", target_bir_lowering=False, debug=False, num_devices=NC
    )

    # ---- dram I/O ----
    d_xT = nc.dram_tensor("xT", [IN, BL * s_steps], B16, kind="ExternalInput")
    d_whhT = [
        nc.dram_tensor(f"whhT{l}", [H, G], B16, kind="ExternalInput")
        for l in range(L)
    ]
    d_wihT = [nc.dram_tensor("wihT0", [IN, G], B16, kind="ExternalInput")] + [
        nc.dram_tensor(f"wihT{l}", [H, G], B16, kind="ExternalInput")
        for l in (1, 2)
    ]
    d_gxbias = [
        nc.dram_tensor(f"gxbias{l}", [128, G], B16, kind="ExternalInput")
        for l in range(L)
    ]
    d_bhhn = [
        nc.dram_tensor(f"bhhn{l}", [1, H], B16, kind="ExternalInput")
        for l in range(L)
    ]
    d_ones = nc.dram_tensor("ones18", [1, BL], B16, kind="ExternalInput")
    d_id8 = nc.dram_tensor("ident8", [BL, BL], B16, kind="ExternalInput")
    d_fcw = nc.dram_tensor("fcwT", [H, T_OUT], B16, kind="ExternalInput")
    d_fcb = nc.dram_tensor("fcb", [1, T_OUT], B16, kind="ExternalInput")
    d_out = nc.dram_tensor("out", [BL, T_OUT], F32, kind="ExternalOutput")

    ntok = BL * s_steps
    nblk = ntok // 128  # 128-token blocks in gx phase

    with tile.TileContext(nc) as tc:
        with (
            tc.tile_pool(name="const", bufs=1) as cpool,
            tc.tile_pool(name="wl", bufs=1) as wlpool,
            tc.tile_pool(name="state", bufs=1) as spool,
            tc.tile_pool(name="gxin", bufs=4) as gxin_pool,
            tc.tile_pool(name="gxout", bufs=2) as gxout_pool,
            tc.tile_pool(name="gates", bufs=2) as gpool,
            tc.tile_pool(name="mm", bufs=2, space="PSUM") as mmpool,
            tc.tile_pool(name="tp", bufs=1, space="PSUM") as tppool,
            tc.tile_pool(name="dm", bufs=1, space="PSUM") as dmpool,
            tc.tile_pool(name="dram", bufs=1, space="DRAM") as dpool,
        ):
            # ---- constants resident all run ----
            xT = cpool.tile([IN, ntok], B16, tag="xT")
            nc.sync.dma_start(xT[:], d_xT.ap())
            ones18 = cpool.tile([1, BL], B16, tag="ones")
            nc.sync.dma_start(ones18[:], d_ones.ap())
            id8 = cpool.tile([BL, BL], B16, tag="id8")
            nc.sync.dma_start(id8[:], d_id8.ap())
            fcw = cpool.tile([128, KC, T_OUT], B16, tag="fcw")
            nc.sync.dma_start(
                fcw[:],
                d_fcw.ap().rearrange("(k p) t -> p k t", p=128),
            )
            fcb = cpool.tile([1, T_OUT], B16, tag="fcb")
            nc.sync.dma_start(fcb[:], d_fcb.ap())

            # history of h^{l}.T for next layer's gx ([128, k, t, b])
            hist = cpool.tile([128, KC, s_steps, BL], B16, tag="hist")

            # persistent state
            curhT = spool.tile([128, KC * BL], B16, tag="curhT")
            h_st = spool.tile([BL, H], B16, tag="h")

            # per-layer weights (reloaded per layer)
            whh = wlpool.tile([128, KC, G], B16, tag="whh")
            wih = wlpool.tile([128, KC, G], B16, tag="wih")
            gxbias = wlpool.tile([128, G], B16, tag="gxbias")
            bhhn = wlpool.tile([1, H], B16, tag="bhhn")

            gx_dram = dpool.tile([ntok, G], B16, tag="gx")
            junk_dram = dpool.tile([BL, 512], B16, tag="junk")

            # HAM filler: the PE idles ~2-3us per step while the gate chain
            # runs, which re-throttles the clock to K=4/8 for the following
            # matmul burst (one oscillation per step, ~2x on every matmul).
            # Dummy accumulating matmuls during the gap keep the activity
            # monitor fed so the real matmuls run at 2.4 GHz.
            dummy_ps = dmpool.tile([128, 512], F32, tag="dm")
            dummy_started = [False]

            def pe_filler(n_mm):
                # full-width (M=128, N=512) matmuls on resident tiles: max
                # array activity per instruction, no data dependencies.
                for i in range(n_mm):
                    nc.tensor.matmul(
                        dummy_ps[:],
                        whh[:, 0, 0:128],
                        gxbias[:, 512:1024],
                        start=(not dummy_started[0]),
                        stop=False,
                        skip_group_check=True,
                    )
                    dummy_started[0] = True

            for layer in range(L):
                # ---------- load layer weights ----------
                nc.sync.dma_start(
                    whh[:],
                    d_whhT[layer].ap().rearrange("(k p) g -> p k g", p=128),
                )
                if layer == 0:
                    nc.sync.dma_start(wih[0:IN, 0, :], d_wihT[0].ap())
                else:
                    nc.sync.dma_start(
                        wih[:],
                        d_wihT[layer].ap().rearrange("(k p) g -> p k g", p=128),
                    )
                nc.sync.dma_start(gxbias[:], d_gxbias[layer].ap())
                nc.sync.dma_start(bhhn[:], d_bhhn[layer].ap())

                # ---------- phase A: gx for all tokens ----------
                kc_in = 1 if layer == 0 else KC
                for blk in range(nblk):
                    gxo = gxout_pool.tile([128, G], B16, tag="gxo")
                    for s3 in range(3):
                        ps = mmpool.tile([128, 512], F32, tag=f"m{s3}")
                        for k in range(kc_in):
                            if layer == 0:
                                lhsT = xT[:, blk * 128 : (blk + 1) * 128]
                                rhs = wih[0:IN, 0, s3 * 512 : (s3 + 1) * 512]
                            else:
                                lhsT = hist[:, k, blk * 16 : (blk + 1) * 16, :]
                                rhs = wih[:, k, s3 * 512 : (s3 + 1) * 512]
                            nc.tensor.matmul(
                                ps[:],
                                lhsT,
                                rhs,
                                start=(k == 0),
                                stop=(k == kc_in - 1),
                            )
                        nc.vector.tensor_add(
                            gxo[:, s3 * 512 : (s3 + 1) * 512],
                            ps[:],
                            gxbias[:, s3 * 512 : (s3 + 1) * 512],
                        )
                    nc.sync.dma_start(
                        gx_dram[blk * 128 : (blk + 1) * 128, :], gxo[:]
                    )

                # ---------- phase B: recurrence ----------
                nc.scalar.memzero(h_st[:])

                def hT_flush(t):
                    # transpose h_{t} (bf16) -> psum, record as stationary
                    # operand for step t+1 and into the gx history.
                    pst = tppool.tile([128, KC * BL], B16, tag="tp")
                    for k in range(KC):
                        nc.tensor.transpose(
                            pst[:, k * BL : (k + 1) * BL],
                            h_st[:, k * 128 : (k + 1) * 128],
                            id8[:],
                        )
                    nc.scalar.copy(curhT[:], pst[:])
                    if layer < L - 1 and t >= 0:
                        nc.vector.tensor_copy(
                            hist[:, :, t, :],
                            pst[:].rearrange("p (k b) -> p k b", k=KC),
                        )

                hT_flush(-1)  # curhT <- 0

                for t in range(s_steps):
                    gxt = gxin_pool.tile([BL, G], B16, tag="gxt")
                    nc.sync.dma_start(
                        gxt[:], gx_dram[t * BL : (t + 1) * BL, :]
                    )
                    ps_r = mmpool.tile([BL, 512], F32, tag="m0")
                    ps_z = mmpool.tile([BL, 512], F32, tag="m1")
                    ps_n = mmpool.tile([BL, 512], F32, tag="m2")
                    # seed psum: r,z tiles get gx (identity matmul), n tile
                    # gets b_hh_n (rank-1).  All independent of h_{t-1}, so
                    # these run inside the previous step's gate-phase gap.
                    nc.tensor.matmul(
                        ps_r[:], id8[:], gxt[:, 0:512],
                        start=True, stop=False, skip_group_check=True,
                    )
                    nc.tensor.matmul(
                        ps_z[:], id8[:], gxt[:, 512:1024],
                        start=True, stop=False, skip_group_check=True,
                    )
                    nc.tensor.matmul(
                        ps_n[:], ones18[:], bhhn[:],
                        start=True, stop=False, skip_group_check=True,
                    )
                    # transpose of h_{t-1} emitted here: in PE program order
                    # the seeds above precede it, filling the PE gap while
                    # step t-1's gates finish.
                    pe_filler(7)
                    if t > 0:
                        hT_flush(t - 1)
                    # gh accumulate, n -> r -> z: t1 needs ps_n and
                    # sigmoid_r, so those finish first; z overlaps t1/t2.
                    for s3, ps in ((2, ps_n), (0, ps_r), (1, ps_z)):
                        for k in range(KC):
                            nc.tensor.matmul(
                                ps[:],
                                curhT[:, k * BL : (k + 1) * BL],
                                whh[:, k, s3 * 512 : (s3 + 1) * 512],
                                start=False,
                                stop=(k == KC - 1),
                                skip_group_check=True,
                            )
                    # gates
                    rz = gpool.tile([BL, 2 * H], B16, tag="rz")
                    nc.scalar.activation(
                        rz[:, 0:H], ps_r[:],
                        mybir.ActivationFunctionType.Sigmoid,
                    )
                    nc.scalar.activation(
                        rz[:, H : 2 * H], ps_z[:],
                        mybir.ActivationFunctionType.Sigmoid,
                    )
                    t1 = gpool.tile([BL, H], B16, tag="t1")
                    nc.vector.tensor_mul(t1[:], rz[:, 0:H], ps_n[:])
                    t2 = gpool.tile([BL, H], B16, tag="t2")
                    nc.vector.tensor_add(t2[:], t1[:], gxt[:, 2 * 512 : 3 * 512])
                    bb = gpool.tile([BL, H], B16, tag="bb")
                    nc.vector.tensor_mul(bb[:], rz[:, H : 2 * H], h_st[:])
                    z2 = gpool.tile([BL, H], B16, tag="z2")
                    nc.gpsimd.tensor_scalar(
                        z2[:], rz[:, H : 2 * H], -1.0, 1.0,
                        mybir.AluOpType.mult, mybir.AluOpType.add,
                    )
                    nt = gpool.tile([BL, H], B16, tag="nt")
                    nc.scalar.activation(
                        nt[:], t2[:], mybir.ActivationFunctionType.Tanh
                    )
                    aa = gpool.tile([BL, H], B16, tag="aa")
                    nc.vector.tensor_mul(aa[:], z2[:], nt[:])
                    nc.vector.tensor_add(h_st[:], aa[:], bb[:])

                # final h.T (records hist[s-1] and leaves curhT = h_final)
                hT_flush(s_steps - 1)

            # anti-DCE reader for the HAM-filler psum: one copy + DMA out
            nc.tensor.matmul(
                dummy_ps[:], whh[:, 0, 0:128], gxbias[:, 0:512],
                start=False, stop=True, skip_group_check=True,
            )
            junk_sb = gpool.tile([BL, 512], B16, tag="junk")
            nc.scalar.copy(junk_sb[:], dummy_ps[0:BL, :])
            nc.sync.dma_start(junk_dram[:], junk_sb[:])

            # ---------- FC head ----------
            psfull = mmpool.tile([BL, 512], F32, tag="m0")
            psf = psfull[:, 0:T_OUT]
            nc.tensor.matmul(
                psf,
                ones18[:],
                fcb[:],
                start=True,
                stop=False,
                skip_group_check=True,
            )
            for k in range(KC):
                nc.tensor.matmul(
                    psf,
                    curhT[:, k * BL : (k + 1) * BL],
                    fcw[:, k, :],
                    start=False,
                    stop=(k == KC - 1),
                    skip_group_check=True,
                )
            out_sb = gpool.tile([BL, T_OUT], F32, tag="osb")
            nc.scalar.copy(out_sb[:], psf)
            nc.sync.dma_start(d_out.ap(), out_sb[:])

    _split_sync_waits(nc)
    return nc


_CACHE = {}


def _get_bass(s_steps):
    if s_steps not in _CACHE:
        _CACHE[s_steps] = build_bass(s_steps)
    return _CACHE[s_steps]


def make_in_maps(inputs, s_steps=S):
    x = np.asarray(inputs["x"], np.float32)
    common = {}
    for l in range(L):
        whh = np.asarray(inputs[f"w_hh_l{l}"], np.float32)  # [G, H]
        common[f"whhT{l}"] = np.ascontiguousarray(whh.T).astype(BF16)
        wih = np.asarray(inputs[f"w_ih_l{l}"], np.float32)  # [G, in]
        common[f"wihT{l}" if l else "wihT0"] = np.ascontiguousarray(
            wih.T
        ).astype(BF16)
        b_ih = np.asarray(inputs[f"b_ih_l{l}"], np.float32)
        b_hh = np.asarray(inputs[f"b_hh_l{l}"], np.float32)
        gb = b_ih.copy()
        gb[: 2 * H] += b_hh[: 2 * H]
        common[f"gxbias{l}"] = np.broadcast_to(
            gb.astype(BF16), (128, G)
        ).copy()
        common[f"bhhn{l}"] = b_hh[2 * H :].reshape(1, H).astype(BF16)
    common["ones18"] = np.ones((1, BL), BF16)
    common["ident8"] = np.eye(BL, dtype=BF16)
    common["fcwT"] = np.ascontiguousarray(
        np.asarray(inputs["fc_w"], np.float32).T
    ).astype(BF16)  # [H, T_OUT]
    common["fcb"] = np.asarray(inputs["fc_b"], np.float32).reshape(
        1, T_OUT
    ).astype(BF16)

    in_maps = []
    for c in range(NC):
        xs = x[c * BL : (c + 1) * BL, :s_steps, :]  # [BL, s, IN]
        xT = np.ascontiguousarray(xs.transpose(2, 1, 0)).reshape(
            IN, s_steps * BL
        )  # col = t*BL+b
        m = dict(common)
        m["xT"] = xT.astype(BF16)
        in_maps.append(m)
    return in_maps


def assemble_output(res) -> np.ndarray:
    out = np.concatenate(
        [res.results[c]["out"] for c in range(NC)], axis=0
    )
    return out.astype(np.float32)


def kernel(**inputs) -> np.ndarray:
    nc = _get_bass(S)
    in_maps = make_in_maps(inputs, S)
    res = run_bass_kernel_spmd(nc, in_maps, core_ids=list(range(NC)))
    return assemble_output(res)
